# revision 20
# baseline (speedup 1.0000x reference)
"""MemNet Trainium2 kernel: 3-hop memory network over embedding gathers.

Data-parallel over batch (16 batches/core x 8 cores).  Host packs the
embedding table into fp8-e4m3 [V, 512] rows (512B, dma_gather-compatible)
split into 4 sub-tables (int16 index reach), and dedupes each core's
32768 token indices per region — attention is permutation/multiplicity
invariant, so unique rows + per-batch multiplicity masks are exact.
Rows are scaled x16 before fp8 quantization; col 300 carries the matching
16.0 "ones" entry so the softmax normalization vec/denominator cancels
the scale exactly, and col 301 carries the host-precomputed unscaled
p = row @ Wa attention projection in fp8.  u0 = mean(emb[targets]) is
host-side (mean commutes with the affine te update, so targets only
enter through u0).

The ~28k unique fp8 rows per core are dma_gather'ed once into 7 group
tiles (32 slots each) and stay resident for all hops.  Gather calls are
1024 rows (2048 hangs the DGE ring) round-robined over 4 SWDGE queues —
single-queue gathers serialize at ~8 ns/row; 4 queues reach ~2.5 ns/row.
Per hop, E = exp(tanh(P + C_b)) * mask is built in batched chunks: the
P+C DVE broadcast-adds for all chunks are emitted up front so the two
Activation passes (tanh, exp) run back-to-back and the PE never stalls
on the DVE chain; the DVE mask multiply converts to fp8.  E is consumed
by fp8 DoubleRow PE matmuls — each contracts a PAIR of slot tiles (256
rows) per pass — accumulating the attention-weighted sum + denominator
in one PSUM tile.  u-updates, c, and the classifier run on transposed u
with host-augmented weights.
"""

import contextlib

import numpy as np

import concourse.bacc as bacc
import concourse.mybir as mybir
import concourse.tile as tile
from concourse.bass_utils import run_bass_kernel_spmd

B, S, T, D, V = 128, 2048, 4, 300, 100000
NCORES, BPC = 8, 16
RSZ = 32768
NREG = 4
DPP = 512         # fp8 row length in bytes/elems (512B, %256)
NE = 301          # vec-matmul out free dim: 300 dims + ones col
PCOL = D + 1      # host-precomputed row@Wa column
ESCALE = 16.0     # fp8 row scale; cancels via the ones column
CH = [(0, 128), (128, 256), (256, 300)]   # d-chunks
WAVE = 8          # slots per gather call (1024 idxs; >=2048 hangs)
SCRATCH = 16384   # SWDGE descriptor ring bytes
SPKT = True       # dma_gather single_packet
GRP = 32          # slots per resident group tile / E-build chunk
F16 = mybir.dt.float16
F32 = mybir.dt.float32
F8 = mybir.dt.float8e4
I16 = mybir.dt.int16
ACT = mybir.ActivationFunctionType
DROW = mybir.MatmulPerfMode.DoubleRow


def _wrap16(loc, cols):
    """int16 index list -> [128, cols] dma_gather layout (16-wrap, 8x repl)."""
    a = np.asarray(loc, np.int16).reshape(cols, 16).T  # [16, cols]
    return np.ascontiguousarray(np.tile(a, (8, 1)))


def _prep(inputs, targets, emb_table, W_att, b_att, W_tr, b_tr, W_out, b_out):
    import ml_dtypes
    F8NP = ml_dtypes.float8_e4m3

    inputs = np.asarray(inputs)
    targets = np.asarray(targets)
    emb_table = np.asarray(emb_table, np.float32)
    W_att = np.asarray(W_att, np.float32).reshape(2 * D)

    tab = np.zeros((V, DPP), F8NP)
    tab[:, :D] = (emb_table * ESCALE).astype(F8NP)
    tab[:, D] = ESCALE
    tab[:, PCOL] = (emb_table @ W_att[:D]).astype(F8NP)
    tabs = [np.ascontiguousarray(tab[r * RSZ:min((r + 1) * RSZ, V)])
            for r in range(NREG)]

    cores = []
    for c in range(NCORES):
        idx = inputs[c * BPC:(c + 1) * BPC].astype(np.int64)  # [16, 2048]
        regs = []
        for r in range(NREG):
            lo, hi = r * RSZ, min((r + 1) * RSZ, V)
            regs.append(np.unique(idx[(idx >= lo) & (idx < hi)]))
        cores.append((idx, regs))
    uslots = [max(max(-(-len(cores[c][1][r]) // 128), 1) for c in range(NCORES))
              for r in range(NREG)]
    if sum(uslots) % 2:
        uslots[-1] += 1          # even slot count for DoubleRow pairing
    sbase = np.concatenate([[0], np.cumsum(uslots)])
    s_slots = int(sbase[-1])

    per_core = []
    for c in range(NCORES):
        idx, regs = cores[c]
        idx16 = []
        lut = np.full(V, -1, np.int64)
        for r in range(NREG):
            u = regs[r]
            n = uslots[r] * 128
            loc = np.zeros(n, np.int64)
            loc[:len(u)] = u - r * RSZ
            idx16.append(_wrap16(loc, n // 16))
            lut[u] = sbase[r] * 128 + np.arange(len(u))
        masks = np.zeros((128, s_slots, BPC), np.float32)
        p = lut[idx].reshape(-1)
        bb = np.repeat(np.arange(BPC), S)
        np.add.at(masks, (p % 128, p // 128, bb), 1.0)

        tgt = targets[c * BPC:(c + 1) * BPC].astype(np.int64)  # [16, 4]
        u0 = emb_table[tgt.reshape(-1)].reshape(BPC, T, D).mean(1)  # [16, D]
        u0T = np.zeros((128, 3, BPC), np.float16)
        for i, (a, b) in enumerate(CH):
            u0T[0:b - a, i, :] = u0[:, a:b].T.astype(np.float16)
        per_core.append(dict(
            idx16=idx16, masks=masks.astype(np.float16), u0T=u0T))

    wuh = np.zeros((128, 3, 1), np.float16)
    for k, (a, b) in enumerate(CH):
        wuh[:b - a, k, 0] = W_att[D + a:D + b].astype(np.float16)
    W_tr = np.asarray(W_tr, np.float32)
    wtrh = np.zeros((128, 3, D), np.float16)
    for j, (a, b) in enumerate(CH):
        wtrh[:b - a, j, :] = W_tr[a:b].astype(np.float16)
    W_out = np.asarray(W_out, np.float32)
    wouth = np.zeros((128, 3, 3), np.float16)
    for j, (a, b) in enumerate(CH):
        wouth[:b - a, j, :] = W_out[a:b].astype(np.float16)
    btrh = np.zeros((128, 3, 1), np.float16)
    for j, (a, b) in enumerate(CH):
        btrh[:b - a, j, 0] = np.asarray(b_tr, np.float32)[a:b].astype(np.float16)
    bouth = np.asarray(b_out, np.float32).reshape(3, 1)
    batth = np.asarray(b_att, np.float32).reshape(1, 1)

    shared = dict(tab0=tabs[0], tab1=tabs[1], tab2=tabs[2], tab3=tabs[3],
                  wuh=wuh, wtrh=wtrh, wouth=wouth, batth=batth,
                  btrh=btrh, bouth=bouth, id16=np.eye(16, dtype=np.float16))
    in_maps = []
    for c in range(NCORES):
        m = dict(shared)
        pc = per_core[c]
        for r in range(NREG):
            m[f"idx{r}"] = pc["idx16"][r]
        m["masks"] = pc["masks"]
        m["u0T"] = pc["u0T"]
        in_maps.append(m)
    meta = dict(uslots=uslots, s_slots=s_slots,
                tabrows=[t.shape[0] for t in tabs])
    return in_maps, meta


def _build(meta, loop_n=None, variant="full", nq=1):
    uslots, s_slots = meta["uslots"], meta["s_slots"]
    bounds = np.concatenate([[0], np.cumsum(uslots)])

    # groups of GRP global slots; each filled by <=WAVE-slot gathers that
    # never cross a region boundary
    groups = []  # (glo, gsz, [(region, local_lo, width, dst_off), ...])
    for glo in range(0, s_slots, GRP):
        gsz = min(GRP, s_slots - glo)
        waves = []
        t = glo
        while t < glo + gsz:
            r = int(np.searchsorted(bounds, t, side="right") - 1)
            w = int(min(WAVE, bounds[r + 1] - t, glo + gsz - t))
            waves.append((r, t - int(bounds[r]), w, t - glo))
            t += w
        groups.append((glo, gsz, waves))

    nc = bacc.Bacc("TRN2", target_bir_lowering=False, num_swdge_queues=nq,
                   dynamic_dma_scratch_size=SCRATCH)
    g = nc.gpsimd

    tabs = [nc.dram_tensor(f"tab{r}", [meta["tabrows"][r], DPP], F8,
                           kind="ExternalInput") for r in range(NREG)]
    idxs = [nc.dram_tensor(f"idx{r}", [128, uslots[r] * 8], I16,
                           kind="ExternalInput") for r in range(NREG)]
    masks_d = nc.dram_tensor("masks", [128, s_slots, BPC], F16,
                             kind="ExternalInput")
    u0T_d = nc.dram_tensor("u0T", [128, 3, BPC], F16, kind="ExternalInput")
    wu_d = nc.dram_tensor("wuh", [128, 3, 1], F16, kind="ExternalInput")
    wtr_d = nc.dram_tensor("wtrh", [128, 3, D], F16, kind="ExternalInput")
    wout_d = nc.dram_tensor("wouth", [128, 3, 3], F16, kind="ExternalInput")
    batt_d = nc.dram_tensor("batth", [1, 1], F32, kind="ExternalInput")
    btr_d = nc.dram_tensor("btrh", [128, 3, 1], F16, kind="ExternalInput")
    bout_d = nc.dram_tensor("bouth", [3, 1], F32, kind="ExternalInput")
    id16_d = nc.dram_tensor("id16", [16, 16], F16, kind="ExternalInput")
    out_d = nc.dram_tensor("outl", [3, BPC], F32, kind="ExternalOutput")

    with tile.TileContext(nc) as tc, contextlib.ExitStack() as ctx:
        const = ctx.enter_context(tc.tile_pool(name="const", bufs=1))
        resp = ctx.enter_context(tc.tile_pool(name="res", bufs=1))
        work = ctx.enter_context(tc.tile_pool(name="work", bufs=2))
        ps = ctx.enter_context(tc.tile_pool(name="ps", bufs=1, space="PSUM"))

        def load(dram, shape, dt, name):
            sb = const.tile(shape, dt, tag=name, name=name + "_sb")
            nc.sync.dma_start(out=sb[:], in_=dram[:])
            return sb
        idx_sb = [load(idxs[r], [128, uslots[r] * 8], I16, f"idxs{r}")
                  for r in range(NREG)]
        u0T_sb = load(u0T_d, [128, 3, BPC], F16, "u0T")
        wu_sb = load(wu_d, [128, 3, 1], F16, "wu")
        wtr_sb = load(wtr_d, [128, 3, D], F16, "wtr")
        wout_sb = load(wout_d, [128, 3, 3], F16, "wout")
        batt_sb = load(batt_d, [1, 1], F32, "batt")
        btr_sb = load(btr_d, [128, 3, 1], F16, "btr")
        bout_sb = load(bout_d, [3, 1], F32, "bout")
        id16_sb = load(id16_d, [16, 16], F16, "id16")
        ones_sb = const.tile([1, 128], F16, tag="onesr", name="onesr")
        nc.vector.memset(ones_sb[:], 1.0)
        masks_sb = load(masks_d, [128, s_slots, BPC], F16, "masks")
        P_sb = const.tile([128, s_slots, 1], F16, tag="P", name="P")

        def body(it):
            def build_C(uT_t, hop):
                cv = ps.tile([1, BPC], F32, tag="mp", bufs=2,
                             name=f"cv{hop}_{it}")
                for k, (a, b) in enumerate(CH):
                    nc.tensor.matmul(cv[:, :], lhsT=wu_sb[0:b - a, k, :],
                                     rhs=uT_t[0:b - a, k, :],
                                     start=(k == 0), stop=(k == 2))
                crow = work.tile([1, BPC], F16, tag="crow",
                                 name=f"crow{hop}_{it}")
                nc.vector.tensor_tensor(
                    out=crow[:], in0=cv[:, :],
                    in1=batt_sb[:].to_broadcast([1, BPC]),
                    op=mybir.AluOpType.add)
                Cp = ps.tile([128, BPC], F32, tag="mp", bufs=2,
                             name=f"Cp{hop}_{it}")
                nc.tensor.matmul(Cp[:, :], lhsT=ones_sb[:], rhs=crow[:],
                                 start=True, stop=True)
                Cm = work.tile([128, 1, BPC], F16, tag="Cm",
                               name=f"Cm{hop}_{it}")
                nc.vector.tensor_copy(Cm[:, 0, :], Cp[:, :])
                return Cm

            def ebuild(E8_t, Cm_t, lo, hi, hop, ci):
                n = hi - lo
                Et = work.tile([128, GRP, BPC], F16, tag="Et", bufs=3,
                               name=f"Et{hop}_{ci}_{it}")
                nc.vector.tensor_tensor(
                    out=Et[:, 0:n, :],
                    in0=P_sb[:, lo:hi, :].to_broadcast([128, n, BPC]),
                    in1=Cm_t[:].to_broadcast([128, n, BPC]),
                    op=mybir.AluOpType.add)
                nc.scalar.activation(Et[:, 0:n, :], Et[:, 0:n, :], ACT.Tanh)
                nc.scalar.activation(Et[:, 0:n, :], Et[:, 0:n, :], ACT.Exp)
                nc.vector.tensor_tensor(
                    out=E8_t[:, lo:hi, :], in0=Et[:, 0:n, :],
                    in1=masks_sb[:, lo:hi, :], op=mybir.AluOpType.mult)

            uT = u0T_sb
            C1 = build_C(uT, 1)

            if variant.startswith("gather"):
                for gi, (glo, gsz, waves) in enumerate(groups):
                    gt = resp.tile([128, gsz, DPP], F8, tag=f"grp{gi}",
                                   name=f"grp{gi}_{it}")
                    for wi, (r, llo, w, off) in enumerate(waves):
                        g.dma_gather(gt[:, off:off + w, :], tabs[r][:],
                                     idx_sb[r][:, llo * 8:(llo + w) * 8],
                                     w * 128, w * 128, DPP,
                                     single_packet=SPKT,
                                     queue_num=(gi * 8 + wi) % nq)
                    if variant == "gather_only":
                        nc.vector.tensor_copy(P_sb[:, glo:glo + gsz, 0],
                                              gt[:, :, PCOL])
                lg_sb = work.tile([3, BPC], F32, tag="lgs", name=f"lgs_{it}")
                nc.vector.tensor_scalar(
                    lg_sb[:], P_sb[0:3, 0:BPC, 0], 1.0, None,
                    mybir.AluOpType.mult)
                nc.sync.dma_start(out=out_d[:], in_=lg_sb[:])
                return

            def wsum(vec_t, E_t, t, gt, off):
                """Weighted-sum matmul(s) for the slot pair (t, t+1)."""
                first, last = t == 0, t == s_slots - 2
                if variant == "nodrow":
                    for dt_ in (0, 1):
                        nc.tensor.matmul(vec_t[:, :],
                                         lhsT=E_t[:, t + dt_, :],
                                         rhs=gt[:, off + dt_, 0:NE],
                                         start=(first and dt_ == 0),
                                         stop=(last and dt_ == 1))
                elif variant == "drow_split":
                    for a, b in ((0, 256), (256, NE)):
                        nc.tensor.matmul(vec_t[:, a:b],
                                         lhsT=E_t[:, t:t + 2, :],
                                         rhs=gt[:, off:off + 2, a:b],
                                         start=first, stop=last,
                                         perf_mode=DROW)
                else:
                    nc.tensor.matmul(vec_t[:, :],
                                     lhsT=E_t[:, t:t + 2, :],
                                     rhs=gt[:, off:off + 2, 0:NE],
                                     start=first, stop=last,
                                     perf_mode=DROW)

            # ---- main pass: group gathers + hop-1 E/PE pipeline ----
            gts = []     # (glo, gsz, tile) for hop-2/3 reuse
            E1 = work.tile([128, s_slots, BPC], F8, tag="E", bufs=2,
                           name=f"E1_{it}")
            vec1 = ps.tile([16, NE], F32, tag="vec", bufs=2, name=f"vec1_{it}")
            for gi, (glo, gsz, waves) in enumerate(groups):
                gt = resp.tile([128, gsz, DPP], F8, tag=f"grp{gi}",
                               bufs=1,
                               name=f"grp{gi}_{it}")
                for wi, (r, llo, w, off) in enumerate(waves):
                    g.dma_gather(gt[:, off:off + w, :], tabs[r][:],
                                 idx_sb[r][:, llo * 8:(llo + w) * 8],
                                 w * 128, w * 128, DPP,
                                 single_packet=SPKT,
                                 queue_num=(gi * 8 + wi) % nq)
                nc.vector.tensor_copy(P_sb[:, glo:glo + gsz, 0],
                                      gt[:, :, PCOL])
                gts.append((glo, gsz, gt))
                ebuild(E1, C1, glo, glo + gsz, 1, gi)
                for j in range(gsz // 2):
                    wsum(vec1, E1, glo + 2 * j, gt, 2 * j)

            # ---- hop tails & remaining hops ----
            def hop_tail(vec, uT_prev, hop):
                zr = work.tile([16, 1], F32, tag="zr", name=f"zr{hop}_{it}")
                nc.vector.reciprocal(zr[:], vec[:, D:D + 1])
                vecN = work.tile([16, NE], F16, tag="vecN",
                                 name=f"vecN{hop}_{it}")
                nc.vector.tensor_scalar(vecN[:], vec[:, :], zr[:], None,
                                        mybir.AluOpType.mult)
                vNT = ps.tile([128, 3, BPC], F16, tag="mp", bufs=2,
                              name=f"vNT{hop}_{it}")
                for i, (a, b) in enumerate(CH):
                    nc.tensor.transpose(vNT[0:b - a, i, :], vecN[:, a:b],
                                        id16_sb[:])
                up = ps.tile([128, 3, BPC], F32, tag="mp", bufs=2,
                             name=f"up{hop}_{it}")
                for i, (a, b) in enumerate(CH):
                    for j, (aj, bj) in enumerate(CH):
                        nc.tensor.matmul(up[0:b - a, i, :],
                                         lhsT=wtr_sb[0:bj - aj, j, a:b],
                                         rhs=uT_prev[0:bj - aj, j, :],
                                         start=(j == 0), stop=(j == 2))
                vNs = work.tile([128, 3, BPC], F16, tag="vNs",
                                name=f"vNs{hop}_{it}")
                for i, (a, b) in enumerate(CH):
                    nc.vector.tensor_copy(vNs[0:b - a, i, :],
                                          vNT[0:b - a, i, :])
                uT_n = work.tile([128, 3, BPC], F16, tag="uT",
                                 name=f"uT{hop}_{it}")
                for i, (a, b) in enumerate(CH):
                    nc.vector.tensor_tensor(
                        out=uT_n[0:b - a, i, :], in0=up[0:b - a, i, :],
                        in1=vNs[0:b - a, i, :], op=mybir.AluOpType.add)
                    nc.vector.tensor_tensor(
                        out=uT_n[0:b - a, i, :], in0=uT_n[0:b - a, i, :],
                        in1=btr_sb[0:b - a, i, :].to_broadcast([b - a, BPC]),
                        op=mybir.AluOpType.add)
                return uT_n

            uT_cur = hop_tail(vec1, uT, 1)
            hops = () if variant == "hop1_only" else (2, 3)
            for hop in hops:
                Cm = build_C(uT_cur, hop)
                E = work.tile([128, s_slots, BPC], F8, tag="E", bufs=2,
                              name=f"E{hop}_{it}")
                vec = ps.tile([16, NE], F32, tag="vec", bufs=2,
                              name=f"vec{hop}_{it}")
                ets = []
                for ci, lo in enumerate(range(0, s_slots, GRP)):
                    hi = min(lo + GRP, s_slots)
                    n = hi - lo
                    Et = work.tile([128, GRP, BPC], F16, tag="Eth", bufs=7,
                                   name=f"Eth{hop}_{ci}_{it}")
                    nc.vector.tensor_tensor(
                        out=Et[:, 0:n, :],
                        in0=P_sb[:, lo:hi, :].to_broadcast([128, n, BPC]),
                        in1=Cm[:].to_broadcast([128, n, BPC]),
                        op=mybir.AluOpType.add)
                    ets.append(Et)
                for ci, lo in enumerate(range(0, s_slots, GRP)):
                    hi = min(lo + GRP, s_slots)
                    n = hi - lo
                    Et = ets[ci]
                    nc.scalar.activation(Et[:, 0:n, :], Et[:, 0:n, :],
                                         ACT.Tanh)
                    nc.scalar.activation(Et[:, 0:n, :], Et[:, 0:n, :],
                                         ACT.Exp)
                    nc.vector.tensor_tensor(
                        out=E[:, lo:hi, :], in0=Et[:, 0:n, :],
                        in1=masks_sb[:, lo:hi, :], op=mybir.AluOpType.mult)
                    for j in range(lo // 2, hi // 2):
                        t = 2 * j
                        gi = t // GRP
                        glo, gsz, gt = gts[gi]
                        wsum(vec, E, t, gt, t - glo)
                uT_cur = hop_tail(vec, uT_cur, hop)

            lg = ps.tile([3, BPC], F32, tag="mp", bufs=2, name=f"lg_{it}")
            for j, (aj, bj) in enumerate(CH):
                nc.tensor.matmul(lg[:, :], lhsT=wout_sb[0:bj - aj, j, :],
                                 rhs=uT_cur[0:bj - aj, j, :],
                                 start=(j == 0), stop=(j == 2))
            lg_sb = work.tile([3, BPC], F32, tag="lgs", name=f"lgs_{it}")
            nc.vector.tensor_tensor(
                out=lg_sb[:], in0=lg[:, :],
                in1=bout_sb[:].to_broadcast([3, BPC]),
                op=mybir.AluOpType.add)
            nc.sync.dma_start(out=out_d[:], in_=lg_sb[:])

        if loop_n is None:
            body(0)
        else:
            with tc.For_i(0, loop_n, 1):
                body(0)
    nc.compile()
    return nc


def kernel(**inputs):
    in_maps, meta = _prep(**inputs)
    nc = _build(meta, nq=4)
    res = run_bass_kernel_spmd(nc, in_maps, core_ids=list(range(NCORES)))
    out = np.zeros((B, 3), np.float32)
    for c in range(NCORES):
        out[c * BPC:(c + 1) * BPC] = res.results[c]["outl"].T
    return out


# revision 22
# speedup vs baseline: 1.0268x; 1.0268x over previous
"""MemNet Trainium2 kernel: 3-hop memory network over embedding gathers.

Data-parallel over batch (16 batches/core x 8 cores).  Host packs the
embedding table into fp8-e4m3 [V, 512] rows (512B, dma_gather-compatible)
split into 4 sub-tables (int16 index reach), and dedupes each core's
32768 token indices per region — attention is permutation/multiplicity
invariant, so unique rows + per-batch multiplicity masks are exact.
Rows are scaled x16 before fp8 quantization; col 300 carries the matching
16.0 "ones" entry so the softmax normalization vec/denominator cancels
the scale exactly, and col 301 carries the host-precomputed unscaled
p = row @ Wa attention projection in fp8.  u0 = mean(emb[targets]) is
host-side (mean commutes with the affine te update, so targets only
enter through u0).

The ~28k unique fp8 rows per core are dma_gather'ed once and stay
resident for all hops.  Gathers are descriptor-rate limited (~8 ns/row
on one SWDGE queue regardless of row size), so (a) calls are
round-robined over 4 SWDGE queues (~2.5 ns/row) and (b) runs of two
CONSECUTIVE vocab ids (~44% of unique rows) are fetched from a host-built
pair table (row i || row i+1, 1024B) at one descriptor per TWO rows —
landing two rows per partition, which is exactly the DoubleRow k-tile
layout the PE consumes.  Calls stay <=1024 descriptors (2048 hangs).
Per hop, E = exp(tanh(P + C_b)) * mask is built in batched chunks: the
P+C DVE broadcast-adds for all chunks are emitted up front so the two
Activation passes (tanh, exp) run back-to-back and the PE never stalls
on the DVE chain; the DVE mask multiply converts to fp8.  E is consumed
by fp8 DoubleRow PE matmuls — each contracts a PAIR of slot tiles (256
rows) per pass — accumulating the attention-weighted sum + denominator
in one PSUM tile.  u-updates, c, and the classifier run on transposed u
with host-augmented weights.
"""

import contextlib

import numpy as np

import concourse.bacc as bacc
import concourse.mybir as mybir
import concourse.tile as tile
from concourse.bass_utils import run_bass_kernel_spmd

B, S, T, D, V = 128, 2048, 4, 300, 100000
NCORES, BPC = 8, 16
RSZ = 32768
NREG = 4
DPP = 512         # fp8 row length in bytes/elems (512B, %256)
NE = 301          # vec-matmul out free dim: 300 dims + ones col
PCOL = D + 1      # host-precomputed row@Wa column
ESCALE = 16.0     # fp8 row scale; cancels via the ones column
CH = [(0, 128), (128, 256), (256, 300)]   # d-chunks
WAVE = 8          # slots per gather call (1024 idxs; >=2048 hangs)
SCRATCH = 16384   # SWDGE descriptor ring bytes
SPKT = True       # dma_gather single_packet
GRP = 32          # slots per resident group tile / E-build chunk
F16 = mybir.dt.float16
F32 = mybir.dt.float32
F8 = mybir.dt.float8e4
I16 = mybir.dt.int16
ACT = mybir.ActivationFunctionType
DROW = mybir.MatmulPerfMode.DoubleRow


def _wrap16(loc, cols):
    """int16 index list -> [128, cols] dma_gather layout (16-wrap, 8x repl)."""
    a = np.asarray(loc, np.int16).reshape(cols, 16).T  # [16, cols]
    return np.ascontiguousarray(np.tile(a, (8, 1)))


def _prep(inputs, targets, emb_table, W_att, b_att, W_tr, b_tr, W_out, b_out):
    import ml_dtypes
    F8NP = ml_dtypes.float8_e4m3

    inputs = np.asarray(inputs)
    targets = np.asarray(targets)
    emb_table = np.asarray(emb_table, np.float32)
    W_att = np.asarray(W_att, np.float32).reshape(2 * D)

    tab = np.zeros((V, DPP), F8NP)
    tab[:, :D] = (emb_table * ESCALE).astype(F8NP)
    tab[:, D] = ESCALE
    tab[:, PCOL] = (emb_table @ W_att[:D]).astype(F8NP)
    tabs = [np.ascontiguousarray(tab[r * RSZ:min((r + 1) * RSZ, V)])
            for r in range(NREG)]
    # pair table: row i = rows (i, i+1) concatenated -> one 1024B descriptor
    # fetches two consecutive embedding rows
    tabps = [np.ascontiguousarray(
        np.concatenate([t[:-1], t[1:]], axis=1)) for t in tabs]

    def split_pairs(u):
        """sorted uniques -> (pair first-values, singles) greedy."""
        pr, sg, i = [], [], 0
        while i < len(u):
            if i + 1 < len(u) and u[i + 1] == u[i] + 1:
                pr.append(u[i]); i += 2
            else:
                sg.append(u[i]); i += 1
        return np.asarray(pr, np.int64), np.asarray(sg, np.int64)

    cores = []
    for c in range(NCORES):
        idx = inputs[c * BPC:(c + 1) * BPC].astype(np.int64)  # [16, 2048]
        regs = []
        for r in range(NREG):
            lo, hi = r * RSZ, min((r + 1) * RSZ, V)
            u = np.unique(idx[(idx >= lo) & (idx < hi)])
            regs.append(split_pairs(u - lo))
        cores.append((idx, regs))
    npb = [max(max(-(-len(cores[c][1][r][0]) // 128), 1) for c in range(NCORES))
           for r in range(NREG)]
    ns = [max(max(-(-len(cores[c][1][r][1]) // 128), 1) for c in range(NCORES))
          for r in range(NREG)]
    ns = [n + (n % 2) for n in ns]     # even single-slots per region
    rslots = [2 * npb[r] + ns[r] for r in range(NREG)]
    rbase = np.concatenate([[0], np.cumsum(rslots)])
    s_slots = int(rbase[-1])

    per_core = []
    for c in range(NCORES):
        idx, regs = cores[c]
        pidx16, sidx16 = [], []
        lut = np.full(V, -1, np.int64)
        for r in range(NREG):
            pr, sg = regs[r]
            rb = int(rbase[r])
            # pairs: pair j -> partition j%128, slots rb+2*(j//128)+{0,1}
            j = np.arange(len(pr))
            lut[pr + r * RSZ] = (rb + 2 * (j // 128)) * 128 + (j % 128)
            lut[pr + 1 + r * RSZ] = (rb + 2 * (j // 128) + 1) * 128 + (j % 128)
            k = np.arange(len(sg))
            lut[sg + r * RSZ] = (rb + 2 * npb[r] + k // 128) * 128 + (k % 128)
            ploc = np.zeros(npb[r] * 128, np.int64)
            ploc[:len(pr)] = pr
            pidx16.append(_wrap16(ploc, npb[r] * 8))
            sloc = np.zeros(ns[r] * 128, np.int64)
            sloc[:len(sg)] = sg
            sidx16.append(_wrap16(sloc, ns[r] * 8))
        masks = np.zeros((128, s_slots, BPC), np.float32)
        p = lut[idx].reshape(-1)
        bb = np.repeat(np.arange(BPC), S)
        np.add.at(masks, (p % 128, p // 128, bb), 1.0)

        tgt = targets[c * BPC:(c + 1) * BPC].astype(np.int64)  # [16, 4]
        u0 = emb_table[tgt.reshape(-1)].reshape(BPC, T, D).mean(1)  # [16, D]
        u0T = np.zeros((128, 3, BPC), np.float16)
        for i, (a, b) in enumerate(CH):
            u0T[0:b - a, i, :] = u0[:, a:b].T.astype(np.float16)
        per_core.append(dict(
            pidx16=pidx16, sidx16=sidx16,
            masks=masks.astype(np.float16), u0T=u0T))

    wuh = np.zeros((128, 3, 1), np.float16)
    for k, (a, b) in enumerate(CH):
        wuh[:b - a, k, 0] = W_att[D + a:D + b].astype(np.float16)
    W_tr = np.asarray(W_tr, np.float32)
    wtrh = np.zeros((128, 3, D), np.float16)
    for j, (a, b) in enumerate(CH):
        wtrh[:b - a, j, :] = W_tr[a:b].astype(np.float16)
    W_out = np.asarray(W_out, np.float32)
    wouth = np.zeros((128, 3, 3), np.float16)
    for j, (a, b) in enumerate(CH):
        wouth[:b - a, j, :] = W_out[a:b].astype(np.float16)
    btrh = np.zeros((128, 3, 1), np.float16)
    for j, (a, b) in enumerate(CH):
        btrh[:b - a, j, 0] = np.asarray(b_tr, np.float32)[a:b].astype(np.float16)
    bouth = np.asarray(b_out, np.float32).reshape(3, 1)
    batth = np.asarray(b_att, np.float32).reshape(1, 1)

    shared = dict(wuh=wuh, wtrh=wtrh, wouth=wouth, batth=batth,
                  btrh=btrh, bouth=bouth, id16=np.eye(16, dtype=np.float16))
    for r in range(NREG):
        shared[f"tab{r}"] = tabs[r]
        shared[f"tabp{r}"] = tabps[r]
    in_maps = []
    for c in range(NCORES):
        m = dict(shared)
        pc = per_core[c]
        for r in range(NREG):
            m[f"pidx{r}"] = pc["pidx16"][r]
            m[f"sidx{r}"] = pc["sidx16"][r]
        m["masks"] = pc["masks"]
        m["u0T"] = pc["u0T"]
        in_maps.append(m)
    meta = dict(npb=npb, ns=ns, s_slots=s_slots,
                tabrows=[t.shape[0] for t in tabs],
                tabprows=[t.shape[0] for t in tabps])
    return in_maps, meta


def _build(meta, loop_n=None, variant="full", nq=1):
    npb, ns, s_slots = meta["npb"], meta["ns"], meta["s_slots"]
    rslots = [2 * npb[r] + ns[r] for r in range(NREG)]
    rbase = np.concatenate([[0], np.cumsum(rslots)])

    # gather calls: ('P', r, blk0, nblk) 1024B pair rows (<=4 blocks = 512
    # descriptors) or ('S', r, s0, w) 512B single rows (<=8 slots)
    calls = []
    for r in range(NREG):
        for b0 in range(0, npb[r], 4):
            calls.append(("P", r, b0, min(4, npb[r] - b0)))
        for s0 in range(0, ns[r], WAVE):
            calls.append(("S", r, s0, min(WAVE, ns[r] - s0)))

    # slot plan for the weighted sums: one DoubleRow per entry
    # ('P', r, blk, slot) or ('S', r, s0, slot) covering slots t, t+1
    plan = []
    for r in range(NREG):
        rb = int(rbase[r])
        for q in range(npb[r]):
            plan.append(("P", r, q, rb + 2 * q))
        for k in range(0, ns[r], 2):
            plan.append(("S", r, k, rb + 2 * npb[r] + k))
    plan.sort(key=lambda e: e[3])

    nc = bacc.Bacc("TRN2", target_bir_lowering=False, num_swdge_queues=nq,
                   dynamic_dma_scratch_size=SCRATCH)
    g = nc.gpsimd

    tabs = [nc.dram_tensor(f"tab{r}", [meta["tabrows"][r], DPP], F8,
                           kind="ExternalInput") for r in range(NREG)]
    tabps = [nc.dram_tensor(f"tabp{r}", [meta["tabprows"][r], 2 * DPP], F8,
                            kind="ExternalInput") for r in range(NREG)]
    pidxs = [nc.dram_tensor(f"pidx{r}", [128, npb[r] * 8], I16,
                            kind="ExternalInput") for r in range(NREG)]
    sidxs = [nc.dram_tensor(f"sidx{r}", [128, ns[r] * 8], I16,
                            kind="ExternalInput") for r in range(NREG)]
    masks_d = nc.dram_tensor("masks", [128, s_slots, BPC], F16,
                             kind="ExternalInput")
    u0T_d = nc.dram_tensor("u0T", [128, 3, BPC], F16, kind="ExternalInput")
    wu_d = nc.dram_tensor("wuh", [128, 3, 1], F16, kind="ExternalInput")
    wtr_d = nc.dram_tensor("wtrh", [128, 3, D], F16, kind="ExternalInput")
    wout_d = nc.dram_tensor("wouth", [128, 3, 3], F16, kind="ExternalInput")
    batt_d = nc.dram_tensor("batth", [1, 1], F32, kind="ExternalInput")
    btr_d = nc.dram_tensor("btrh", [128, 3, 1], F16, kind="ExternalInput")
    bout_d = nc.dram_tensor("bouth", [3, 1], F32, kind="ExternalInput")
    id16_d = nc.dram_tensor("id16", [16, 16], F16, kind="ExternalInput")
    out_d = nc.dram_tensor("outl", [3, BPC], F32, kind="ExternalOutput")

    with tile.TileContext(nc) as tc, contextlib.ExitStack() as ctx:
        const = ctx.enter_context(tc.tile_pool(name="const", bufs=1))
        resp = ctx.enter_context(tc.tile_pool(name="res", bufs=1))
        work = ctx.enter_context(tc.tile_pool(name="work", bufs=2))
        ps = ctx.enter_context(tc.tile_pool(name="ps", bufs=1, space="PSUM"))

        def load(dram, shape, dt, name):
            sb = const.tile(shape, dt, tag=name, name=name + "_sb")
            nc.sync.dma_start(out=sb[:], in_=dram[:])
            return sb
        pidx_sb = [load(pidxs[r], [128, npb[r] * 8], I16, f"pidxs{r}")
                   for r in range(NREG)]
        sidx_sb = [load(sidxs[r], [128, ns[r] * 8], I16, f"sidxs{r}")
                   for r in range(NREG)]
        u0T_sb = load(u0T_d, [128, 3, BPC], F16, "u0T")
        wu_sb = load(wu_d, [128, 3, 1], F16, "wu")
        wtr_sb = load(wtr_d, [128, 3, D], F16, "wtr")
        wout_sb = load(wout_d, [128, 3, 3], F16, "wout")
        batt_sb = load(batt_d, [1, 1], F32, "batt")
        btr_sb = load(btr_d, [128, 3, 1], F16, "btr")
        bout_sb = load(bout_d, [3, 1], F32, "bout")
        id16_sb = load(id16_d, [16, 16], F16, "id16")
        ones_sb = const.tile([1, 128], F16, tag="onesr", name="onesr")
        nc.vector.memset(ones_sb[:], 1.0)
        masks_sb = load(masks_d, [128, s_slots, BPC], F16, "masks")
        P_sb = const.tile([128, s_slots, 1], F16, tag="P", name="P")

        def body(it):
            def build_C(uT_t, hop):
                cv = ps.tile([1, BPC], F32, tag="mp", bufs=2,
                             name=f"cv{hop}_{it}")
                for k, (a, b) in enumerate(CH):
                    nc.tensor.matmul(cv[:, :], lhsT=wu_sb[0:b - a, k, :],
                                     rhs=uT_t[0:b - a, k, :],
                                     start=(k == 0), stop=(k == 2))
                crow = work.tile([1, BPC], F16, tag="crow",
                                 name=f"crow{hop}_{it}")
                nc.vector.tensor_tensor(
                    out=crow[:], in0=cv[:, :],
                    in1=batt_sb[:].to_broadcast([1, BPC]),
                    op=mybir.AluOpType.add)
                Cp = ps.tile([128, BPC], F32, tag="mp", bufs=2,
                             name=f"Cp{hop}_{it}")
                nc.tensor.matmul(Cp[:, :], lhsT=ones_sb[:], rhs=crow[:],
                                 start=True, stop=True)
                Cm = work.tile([128, 1, BPC], F16, tag="Cm",
                               name=f"Cm{hop}_{it}")
                nc.vector.tensor_copy(Cm[:, 0, :], Cp[:, :])
                return Cm

            def ebuild(E8_t, Cm_t, lo, hi, hop, ci):
                n = hi - lo
                Et = work.tile([128, GRP, BPC], F16, tag="Et", bufs=3,
                               name=f"Et{hop}_{ci}_{it}")
                nc.vector.tensor_tensor(
                    out=Et[:, 0:n, :],
                    in0=P_sb[:, lo:hi, :].to_broadcast([128, n, BPC]),
                    in1=Cm_t[:].to_broadcast([128, n, BPC]),
                    op=mybir.AluOpType.add)
                nc.scalar.activation(Et[:, 0:n, :], Et[:, 0:n, :], ACT.Tanh)
                nc.scalar.activation(Et[:, 0:n, :], Et[:, 0:n, :], ACT.Exp)
                nc.vector.tensor_tensor(
                    out=E8_t[:, lo:hi, :], in0=Et[:, 0:n, :],
                    in1=masks_sb[:, lo:hi, :], op=mybir.AluOpType.mult)

            uT = u0T_sb
            C1 = build_C(uT, 1)

            ptiles = [resp.tile([128, npb[r], 2 * DPP], F8, tag=f"tp{r}",
                                name=f"tp{r}_{it}") for r in range(NREG)]
            stiles = [resp.tile([128, ns[r], DPP], F8, tag=f"ts{r}",
                                name=f"ts{r}_{it}") for r in range(NREG)]

            def emit_gathers(with_pcopy=True):
                qload = [0] * nq
                for ci_, (kind, r, a, w) in enumerate(calls):
                    q = min(range(nq), key=lambda i: qload[i])
                    qload[q] += w * 128
                    rb = int(rbase[r])
                    if kind == "P":
                        tp = ptiles[r]
                        g.dma_gather(tp[:, a:a + w, :], tabps[r][:],
                                     pidx_sb[r][:, a * 8:(a + w) * 8],
                                     w * 128, w * 128, 2 * DPP,
                                     single_packet=SPKT, queue_num=q)
                        if with_pcopy:
                            pv = tp[:, a:a + w, :].rearrange(
                                "p q (m d) -> p q m d", m=2)[:, :, :, PCOL]
                            t0 = rb + 2 * a
                            nc.vector.tensor_copy(
                                P_sb[:, t0:t0 + 2 * w, 0], pv)
                    else:
                        ts = stiles[r]
                        g.dma_gather(ts[:, a:a + w, :], tabs[r][:],
                                     sidx_sb[r][:, a * 8:(a + w) * 8],
                                     w * 128, w * 128, DPP,
                                     single_packet=SPKT, queue_num=q)
                        if with_pcopy:
                            t0 = rb + 2 * npb[r] + a
                            nc.vector.tensor_copy(
                                P_sb[:, t0:t0 + w, 0], ts[:, a:a + w, PCOL])

            if variant == "gather_only":
                emit_gathers()
                lg_sb = work.tile([3, BPC], F32, tag="lgs", name=f"lgs_{it}")
                nc.vector.tensor_scalar(
                    lg_sb[:], P_sb[0:3, 0:BPC, 0], 1.0, None,
                    mybir.AluOpType.mult)
                nc.sync.dma_start(out=out_d[:], in_=lg_sb[:])
                return

            def wsum(vec_t, E_t, entry, pi):
                """DoubleRow weighted sum for plan entry (slots t, t+1)."""
                kind, r, a, t = entry
                first, last = pi == 0, pi == len(plan) - 1
                if kind == "P":
                    rhs = ptiles[r][:, a, :].rearrange(
                        "p (m d) -> p m d", m=2)[:, :, 0:NE]
                else:
                    rhs = stiles[r][:, a:a + 2, 0:NE]
                nc.tensor.matmul(vec_t[:, :], lhsT=E_t[:, t:t + 2, :],
                                 rhs=rhs, start=first, stop=last,
                                 perf_mode=DROW)

            # ---- main pass: gathers + hop-1 E/PE pipeline ----
            E1 = work.tile([128, s_slots, BPC], F8, tag="E", bufs=2,
                           name=f"E1_{it}")
            vec1 = ps.tile([16, NE], F32, tag="vec", bufs=2, name=f"vec1_{it}")
            emit_gathers()
            pi = 0
            for ci_, lo in enumerate(range(0, s_slots, GRP)):
                hi = min(lo + GRP, s_slots)
                ebuild(E1, C1, lo, hi, 1, ci_)
                while pi < len(plan) and plan[pi][3] + 2 <= hi:
                    wsum(vec1, E1, plan[pi], pi)
                    pi += 1
            assert pi == len(plan)

            # ---- hop tails & remaining hops ----
            def hop_tail(vec, uT_prev, hop):
                zr = work.tile([16, 1], F32, tag="zr", name=f"zr{hop}_{it}")
                nc.vector.reciprocal(zr[:], vec[:, D:D + 1])
                vecN = work.tile([16, NE], F16, tag="vecN",
                                 name=f"vecN{hop}_{it}")
                nc.vector.tensor_scalar(vecN[:], vec[:, :], zr[:], None,
                                        mybir.AluOpType.mult)
                vNT = ps.tile([128, 3, BPC], F16, tag="mp", bufs=2,
                              name=f"vNT{hop}_{it}")
                for i, (a, b) in enumerate(CH):
                    nc.tensor.transpose(vNT[0:b - a, i, :], vecN[:, a:b],
                                        id16_sb[:])
                up = ps.tile([128, 3, BPC], F32, tag="mp", bufs=2,
                             name=f"up{hop}_{it}")
                for i, (a, b) in enumerate(CH):
                    for j, (aj, bj) in enumerate(CH):
                        nc.tensor.matmul(up[0:b - a, i, :],
                                         lhsT=wtr_sb[0:bj - aj, j, a:b],
                                         rhs=uT_prev[0:bj - aj, j, :],
                                         start=(j == 0), stop=(j == 2))
                vNs = work.tile([128, 3, BPC], F16, tag="vNs",
                                name=f"vNs{hop}_{it}")
                for i, (a, b) in enumerate(CH):
                    nc.vector.tensor_copy(vNs[0:b - a, i, :],
                                          vNT[0:b - a, i, :])
                uT_n = work.tile([128, 3, BPC], F16, tag="uT",
                                 name=f"uT{hop}_{it}")
                for i, (a, b) in enumerate(CH):
                    nc.vector.tensor_tensor(
                        out=uT_n[0:b - a, i, :], in0=up[0:b - a, i, :],
                        in1=vNs[0:b - a, i, :], op=mybir.AluOpType.add)
                    nc.vector.tensor_tensor(
                        out=uT_n[0:b - a, i, :], in0=uT_n[0:b - a, i, :],
                        in1=btr_sb[0:b - a, i, :].to_broadcast([b - a, BPC]),
                        op=mybir.AluOpType.add)
                return uT_n

            uT_cur = hop_tail(vec1, uT, 1)
            hops = () if variant == "hop1_only" else (2, 3)
            for hop in hops:
                Cm = build_C(uT_cur, hop)
                E = work.tile([128, s_slots, BPC], F8, tag="E", bufs=2,
                              name=f"E{hop}_{it}")
                vec = ps.tile([16, NE], F32, tag="vec", bufs=2,
                              name=f"vec{hop}_{it}")
                ets = []
                ECH = 56
                nch = -(-s_slots // ECH)
                for ci, lo in enumerate(range(0, s_slots, ECH)):
                    hi = min(lo + ECH, s_slots)
                    n = hi - lo
                    Et = work.tile([128, ECH, BPC], F16, tag="Eth", bufs=nch,
                                   name=f"Eth{hop}_{ci}_{it}")
                    nc.vector.tensor_tensor(
                        out=Et[:, 0:n, :],
                        in0=P_sb[:, lo:hi, :].to_broadcast([128, n, BPC]),
                        in1=Cm[:].to_broadcast([128, n, BPC]),
                        op=mybir.AluOpType.add)
                    ets.append(Et)
                pi = 0
                for ci, lo in enumerate(range(0, s_slots, ECH)):
                    hi = min(lo + ECH, s_slots)
                    n = hi - lo
                    Et = ets[ci]
                    nc.scalar.activation(Et[:, 0:n, :], Et[:, 0:n, :],
                                         ACT.Tanh)
                    nc.scalar.activation(Et[:, 0:n, :], Et[:, 0:n, :],
                                         ACT.Exp)
                    nc.vector.tensor_tensor(
                        out=E[:, lo:hi, :], in0=Et[:, 0:n, :],
                        in1=masks_sb[:, lo:hi, :], op=mybir.AluOpType.mult)
                    while pi < len(plan) and plan[pi][3] + 2 <= hi:
                        wsum(vec, E, plan[pi], pi)
                        pi += 1
                uT_cur = hop_tail(vec, uT_cur, hop)

            lg = ps.tile([3, BPC], F32, tag="mp", bufs=2, name=f"lg_{it}")
            for j, (aj, bj) in enumerate(CH):
                nc.tensor.matmul(lg[:, :], lhsT=wout_sb[0:bj - aj, j, :],
                                 rhs=uT_cur[0:bj - aj, j, :],
                                 start=(j == 0), stop=(j == 2))
            lg_sb = work.tile([3, BPC], F32, tag="lgs", name=f"lgs_{it}")
            nc.vector.tensor_tensor(
                out=lg_sb[:], in0=lg[:, :],
                in1=bout_sb[:].to_broadcast([3, BPC]),
                op=mybir.AluOpType.add)
            nc.sync.dma_start(out=out_d[:], in_=lg_sb[:])

        if loop_n is None:
            body(0)
        else:
            with tc.For_i(0, loop_n, 1):
                body(0)
    nc.compile()
    return nc


def kernel(**inputs):
    in_maps, meta = _prep(**inputs)
    nc = _build(meta, nq=4)
    res = run_bass_kernel_spmd(nc, in_maps, core_ids=list(range(NCORES)))
    out = np.zeros((B, 3), np.float32)
    for c in range(NCORES):
        out[c * BPC:(c + 1) * BPC] = res.results[c]["outl"].T
    return out


# revision 23
# speedup vs baseline: 1.0334x; 1.0064x over previous
"""MemNet Trainium2 kernel: 3-hop memory network over embedding gathers.

Data-parallel over batch (16 batches/core x 8 cores).  Host packs the
embedding table into fp8-e4m3 [V, 512] rows (512B, dma_gather-compatible)
split into 4 sub-tables (int16 index reach), and dedupes each core's
32768 token indices per region — attention is permutation/multiplicity
invariant, so unique rows + per-batch multiplicity masks are exact.
Rows are scaled x16 before fp8 quantization; col 300 carries the matching
16.0 "ones" entry so the softmax normalization vec/denominator cancels
the scale exactly, and col 301 carries the host-precomputed unscaled
p = row @ Wa attention projection in fp8.  u0 = mean(emb[targets]) is
host-side (mean commutes with the affine te update, so targets only
enter through u0).

The ~28k unique fp8 rows per core are dma_gather'ed once and stay
resident for all hops.  Gathers are descriptor-rate limited (~8 ns/row
on one SWDGE queue regardless of row size), so (a) calls are
round-robined over 4 SWDGE queues (~2.5 ns/row) and (b) runs of two
CONSECUTIVE vocab ids (~44% of unique rows) are fetched from a host-built
pair table (rows i and i+1 packed at 384B stride into 768B rows — the
row payload is only 302B) at one descriptor per TWO rows and 25% fewer
pair bytes (the 4-queue gather is HBM-byte-bound) —
landing two rows per partition, which is exactly the DoubleRow k-tile
layout the PE consumes.  Calls stay <=1024 descriptors (2048 hangs).
Per hop, E = exp(tanh(P + C_b)) * mask is built in batched chunks: the
P+C DVE broadcast-adds for all chunks are emitted up front so the two
Activation passes (tanh, exp) run back-to-back and the PE never stalls
on the DVE chain; the DVE mask multiply converts to fp8.  E is consumed
by fp8 DoubleRow PE matmuls — each contracts a PAIR of slot tiles (256
rows) per pass — accumulating the attention-weighted sum + denominator
in one PSUM tile.  u-updates, c, and the classifier run on transposed u
with host-augmented weights.
"""

import contextlib

import numpy as np

import concourse.bacc as bacc
import concourse.mybir as mybir
import concourse.tile as tile
from concourse.bass_utils import run_bass_kernel_spmd

B, S, T, D, V = 128, 2048, 4, 300, 100000
NCORES, BPC = 8, 16
RSZ = 32768
NREG = 4
DPP = 512         # fp8 row length in bytes/elems (512B, %256)
NE = 301          # vec-matmul out free dim: 300 dims + ones col
PCOL = D + 1      # host-precomputed row@Wa column
ESCALE = 16.0     # fp8 row scale; cancels via the ones column
CH = [(0, 128), (128, 256), (256, 300)]   # d-chunks
WAVE = 8          # slots per gather call (1024 idxs; >=2048 hangs)
SCRATCH = 16384   # SWDGE descriptor ring bytes
SPKT = True       # dma_gather single_packet
GRP = 32          # slots per resident group tile / E-build chunk
PDP = 768         # packed pair-row bytes (two rows at 384B stride)
PSTRIDE = 384     # member stride inside a pair row
F16 = mybir.dt.float16
F32 = mybir.dt.float32
F8 = mybir.dt.float8e4
I16 = mybir.dt.int16
ACT = mybir.ActivationFunctionType
DROW = mybir.MatmulPerfMode.DoubleRow


def _wrap16(loc, cols):
    """int16 index list -> [128, cols] dma_gather layout (16-wrap, 8x repl)."""
    a = np.asarray(loc, np.int16).reshape(cols, 16).T  # [16, cols]
    return np.ascontiguousarray(np.tile(a, (8, 1)))


def _prep(inputs, targets, emb_table, W_att, b_att, W_tr, b_tr, W_out, b_out):
    import ml_dtypes
    F8NP = ml_dtypes.float8_e4m3

    inputs = np.asarray(inputs)
    targets = np.asarray(targets)
    emb_table = np.asarray(emb_table, np.float32)
    W_att = np.asarray(W_att, np.float32).reshape(2 * D)

    tab = np.zeros((V, DPP), F8NP)
    tab[:, :D] = (emb_table * ESCALE).astype(F8NP)
    tab[:, D] = ESCALE
    tab[:, PCOL] = (emb_table @ W_att[:D]).astype(F8NP)
    tabs = [np.ascontiguousarray(tab[r * RSZ:min((r + 1) * RSZ, V)])
            for r in range(NREG)]
    # pair table: row i = rows (i, i+1) packed at 384B stride (row payload
    # is <=302B) -> one 768B descriptor fetches two consecutive rows
    tabps = []
    for t in tabs:
        tp = np.zeros((t.shape[0] - 1, PDP), t.dtype)
        tp[:, 0:PSTRIDE] = t[:-1, 0:PSTRIDE]
        tp[:, PSTRIDE:2 * PSTRIDE] = t[1:, 0:PSTRIDE]
        tabps.append(np.ascontiguousarray(tp))

    def split_pairs(u):
        """sorted uniques -> (pair first-values, singles) greedy."""
        pr, sg, i = [], [], 0
        while i < len(u):
            if i + 1 < len(u) and u[i + 1] == u[i] + 1:
                pr.append(u[i]); i += 2
            else:
                sg.append(u[i]); i += 1
        return np.asarray(pr, np.int64), np.asarray(sg, np.int64)

    cores = []
    for c in range(NCORES):
        idx = inputs[c * BPC:(c + 1) * BPC].astype(np.int64)  # [16, 2048]
        regs = []
        for r in range(NREG):
            lo, hi = r * RSZ, min((r + 1) * RSZ, V)
            u = np.unique(idx[(idx >= lo) & (idx < hi)])
            regs.append(split_pairs(u - lo))
        cores.append((idx, regs))
    npb = [max(max(-(-len(cores[c][1][r][0]) // 128), 1) for c in range(NCORES))
           for r in range(NREG)]
    ns = [max(max(-(-len(cores[c][1][r][1]) // 128), 1) for c in range(NCORES))
          for r in range(NREG)]
    ns = [n + (n % 2) for n in ns]     # even single-slots per region
    rslots = [2 * npb[r] + ns[r] for r in range(NREG)]
    rbase = np.concatenate([[0], np.cumsum(rslots)])
    s_slots = int(rbase[-1])

    per_core = []
    for c in range(NCORES):
        idx, regs = cores[c]
        pidx16, sidx16 = [], []
        lut = np.full(V, -1, np.int64)
        for r in range(NREG):
            pr, sg = regs[r]
            rb = int(rbase[r])
            # pairs: pair j -> partition j%128, slots rb+2*(j//128)+{0,1}
            j = np.arange(len(pr))
            lut[pr + r * RSZ] = (rb + 2 * (j // 128)) * 128 + (j % 128)
            lut[pr + 1 + r * RSZ] = (rb + 2 * (j // 128) + 1) * 128 + (j % 128)
            k = np.arange(len(sg))
            lut[sg + r * RSZ] = (rb + 2 * npb[r] + k // 128) * 128 + (k % 128)
            ploc = np.zeros(npb[r] * 128, np.int64)
            ploc[:len(pr)] = pr
            pidx16.append(_wrap16(ploc, npb[r] * 8))
            sloc = np.zeros(ns[r] * 128, np.int64)
            sloc[:len(sg)] = sg
            sidx16.append(_wrap16(sloc, ns[r] * 8))
        masks = np.zeros((128, s_slots, BPC), np.float32)
        p = lut[idx].reshape(-1)
        bb = np.repeat(np.arange(BPC), S)
        np.add.at(masks, (p % 128, p // 128, bb), 1.0)

        tgt = targets[c * BPC:(c + 1) * BPC].astype(np.int64)  # [16, 4]
        u0 = emb_table[tgt.reshape(-1)].reshape(BPC, T, D).mean(1)  # [16, D]
        u0T = np.zeros((128, 3, BPC), np.float16)
        for i, (a, b) in enumerate(CH):
            u0T[0:b - a, i, :] = u0[:, a:b].T.astype(np.float16)
        per_core.append(dict(
            pidx16=pidx16, sidx16=sidx16,
            masks=masks.astype(np.float16), u0T=u0T))

    wuh = np.zeros((128, 3, 1), np.float16)
    for k, (a, b) in enumerate(CH):
        wuh[:b - a, k, 0] = W_att[D + a:D + b].astype(np.float16)
    W_tr = np.asarray(W_tr, np.float32)
    wtrh = np.zeros((128, 3, D), np.float16)
    for j, (a, b) in enumerate(CH):
        wtrh[:b - a, j, :] = W_tr[a:b].astype(np.float16)
    W_out = np.asarray(W_out, np.float32)
    wouth = np.zeros((128, 3, 3), np.float16)
    for j, (a, b) in enumerate(CH):
        wouth[:b - a, j, :] = W_out[a:b].astype(np.float16)
    btrh = np.zeros((128, 3, 1), np.float16)
    for j, (a, b) in enumerate(CH):
        btrh[:b - a, j, 0] = np.asarray(b_tr, np.float32)[a:b].astype(np.float16)
    bouth = np.asarray(b_out, np.float32).reshape(3, 1)
    batth = np.asarray(b_att, np.float32).reshape(1, 1)

    shared = dict(wuh=wuh, wtrh=wtrh, wouth=wouth, batth=batth,
                  btrh=btrh, bouth=bouth, id16=np.eye(16, dtype=np.float16))
    for r in range(NREG):
        shared[f"tab{r}"] = tabs[r]
        shared[f"tabp{r}"] = tabps[r]
    in_maps = []
    for c in range(NCORES):
        m = dict(shared)
        pc = per_core[c]
        for r in range(NREG):
            m[f"pidx{r}"] = pc["pidx16"][r]
            m[f"sidx{r}"] = pc["sidx16"][r]
        m["masks"] = pc["masks"]
        m["u0T"] = pc["u0T"]
        in_maps.append(m)
    meta = dict(npb=npb, ns=ns, s_slots=s_slots,
                tabrows=[t.shape[0] for t in tabs],
                tabprows=[t.shape[0] for t in tabps])
    return in_maps, meta


def _build(meta, loop_n=None, variant="full", nq=1):
    npb, ns, s_slots = meta["npb"], meta["ns"], meta["s_slots"]
    rslots = [2 * npb[r] + ns[r] for r in range(NREG)]
    rbase = np.concatenate([[0], np.cumsum(rslots)])

    # gather calls: ('P', r, blk0, nblk) 1024B pair rows (<=4 blocks = 512
    # descriptors) or ('S', r, s0, w) 512B single rows (<=8 slots)
    calls = []
    for r in range(NREG):
        for b0 in range(0, npb[r], 4):
            calls.append(("P", r, b0, min(4, npb[r] - b0)))
        for s0 in range(0, ns[r], WAVE):
            calls.append(("S", r, s0, min(WAVE, ns[r] - s0)))

    # slot plan for the weighted sums: one DoubleRow per entry
    # ('P', r, blk, slot) or ('S', r, s0, slot) covering slots t, t+1
    plan = []
    for r in range(NREG):
        rb = int(rbase[r])
        for q in range(npb[r]):
            plan.append(("P", r, q, rb + 2 * q))
        for k in range(0, ns[r], 2):
            plan.append(("S", r, k, rb + 2 * npb[r] + k))
    plan.sort(key=lambda e: e[3])

    nc = bacc.Bacc("TRN2", target_bir_lowering=False, num_swdge_queues=nq,
                   dynamic_dma_scratch_size=SCRATCH)
    g = nc.gpsimd

    tabs = [nc.dram_tensor(f"tab{r}", [meta["tabrows"][r], DPP], F8,
                           kind="ExternalInput") for r in range(NREG)]
    tabps = [nc.dram_tensor(f"tabp{r}", [meta["tabprows"][r], PDP], F8,
                            kind="ExternalInput") for r in range(NREG)]
    pidxs = [nc.dram_tensor(f"pidx{r}", [128, npb[r] * 8], I16,
                            kind="ExternalInput") for r in range(NREG)]
    sidxs = [nc.dram_tensor(f"sidx{r}", [128, ns[r] * 8], I16,
                            kind="ExternalInput") for r in range(NREG)]
    masks_d = nc.dram_tensor("masks", [128, s_slots, BPC], F16,
                             kind="ExternalInput")
    u0T_d = nc.dram_tensor("u0T", [128, 3, BPC], F16, kind="ExternalInput")
    wu_d = nc.dram_tensor("wuh", [128, 3, 1], F16, kind="ExternalInput")
    wtr_d = nc.dram_tensor("wtrh", [128, 3, D], F16, kind="ExternalInput")
    wout_d = nc.dram_tensor("wouth", [128, 3, 3], F16, kind="ExternalInput")
    batt_d = nc.dram_tensor("batth", [1, 1], F32, kind="ExternalInput")
    btr_d = nc.dram_tensor("btrh", [128, 3, 1], F16, kind="ExternalInput")
    bout_d = nc.dram_tensor("bouth", [3, 1], F32, kind="ExternalInput")
    id16_d = nc.dram_tensor("id16", [16, 16], F16, kind="ExternalInput")
    out_d = nc.dram_tensor("outl", [3, BPC], F32, kind="ExternalOutput")

    with tile.TileContext(nc) as tc, contextlib.ExitStack() as ctx:
        const = ctx.enter_context(tc.tile_pool(name="const", bufs=1))
        resp = ctx.enter_context(tc.tile_pool(name="res", bufs=1))
        work = ctx.enter_context(tc.tile_pool(name="work", bufs=2))
        ps = ctx.enter_context(tc.tile_pool(name="ps", bufs=1, space="PSUM"))

        def load(dram, shape, dt, name):
            sb = const.tile(shape, dt, tag=name, name=name + "_sb")
            nc.sync.dma_start(out=sb[:], in_=dram[:])
            return sb
        pidx_sb = [load(pidxs[r], [128, npb[r] * 8], I16, f"pidxs{r}")
                   for r in range(NREG)]
        sidx_sb = [load(sidxs[r], [128, ns[r] * 8], I16, f"sidxs{r}")
                   for r in range(NREG)]
        u0T_sb = load(u0T_d, [128, 3, BPC], F16, "u0T")
        wu_sb = load(wu_d, [128, 3, 1], F16, "wu")
        wtr_sb = load(wtr_d, [128, 3, D], F16, "wtr")
        wout_sb = load(wout_d, [128, 3, 3], F16, "wout")
        batt_sb = load(batt_d, [1, 1], F32, "batt")
        btr_sb = load(btr_d, [128, 3, 1], F16, "btr")
        bout_sb = load(bout_d, [3, 1], F32, "bout")
        id16_sb = load(id16_d, [16, 16], F16, "id16")
        ones_sb = const.tile([1, 128], F16, tag="onesr", name="onesr")
        nc.vector.memset(ones_sb[:], 1.0)
        masks_sb = load(masks_d, [128, s_slots, BPC], F16, "masks")
        P_sb = const.tile([128, s_slots, 1], F16, tag="P", name="P")

        def body(it):
            def build_C(uT_t, hop):
                cv = ps.tile([1, BPC], F32, tag="mp", bufs=2,
                             name=f"cv{hop}_{it}")
                for k, (a, b) in enumerate(CH):
                    nc.tensor.matmul(cv[:, :], lhsT=wu_sb[0:b - a, k, :],
                                     rhs=uT_t[0:b - a, k, :],
                                     start=(k == 0), stop=(k == 2))
                crow = work.tile([1, BPC], F16, tag="crow",
                                 name=f"crow{hop}_{it}")
                nc.vector.tensor_tensor(
                    out=crow[:], in0=cv[:, :],
                    in1=batt_sb[:].to_broadcast([1, BPC]),
                    op=mybir.AluOpType.add)
                Cp = ps.tile([128, BPC], F32, tag="mp", bufs=2,
                             name=f"Cp{hop}_{it}")
                nc.tensor.matmul(Cp[:, :], lhsT=ones_sb[:], rhs=crow[:],
                                 start=True, stop=True)
                Cm = work.tile([128, 1, BPC], F16, tag="Cm",
                               name=f"Cm{hop}_{it}")
                nc.vector.tensor_copy(Cm[:, 0, :], Cp[:, :])
                return Cm

            def ebuild(E8_t, Cm_t, lo, hi, hop, ci):
                n = hi - lo
                Et = work.tile([128, GRP, BPC], F16, tag="Et", bufs=3,
                               name=f"Et{hop}_{ci}_{it}")
                nc.vector.tensor_tensor(
                    out=Et[:, 0:n, :],
                    in0=P_sb[:, lo:hi, :].to_broadcast([128, n, BPC]),
                    in1=Cm_t[:].to_broadcast([128, n, BPC]),
                    op=mybir.AluOpType.add)
                nc.scalar.activation(Et[:, 0:n, :], Et[:, 0:n, :], ACT.Tanh)
                nc.scalar.activation(Et[:, 0:n, :], Et[:, 0:n, :], ACT.Exp)
                nc.vector.tensor_tensor(
                    out=E8_t[:, lo:hi, :], in0=Et[:, 0:n, :],
                    in1=masks_sb[:, lo:hi, :], op=mybir.AluOpType.mult)

            uT = u0T_sb
            C1 = build_C(uT, 1)

            ptiles = [resp.tile([128, npb[r], PDP], F8, tag=f"tp{r}",
                                name=f"tp{r}_{it}") for r in range(NREG)]
            stiles = [resp.tile([128, ns[r], DPP], F8, tag=f"ts{r}",
                                name=f"ts{r}_{it}") for r in range(NREG)]

            def emit_gathers(with_pcopy=True):
                qload = [0] * nq
                for ci_, (kind, r, a, w) in enumerate(calls):
                    q = min(range(nq), key=lambda i: qload[i])
                    qload[q] += w * 128
                    rb = int(rbase[r])
                    if kind == "P":
                        tp = ptiles[r]
                        g.dma_gather(tp[:, a:a + w, :], tabps[r][:],
                                     pidx_sb[r][:, a * 8:(a + w) * 8],
                                     w * 128, w * 128, PDP,
                                     single_packet=SPKT, queue_num=q)
                        if with_pcopy:
                            pv = tp[:, a:a + w, :].rearrange(
                                "p q (m d) -> p q m d", m=2)[:, :, :, PCOL]
                            t0 = rb + 2 * a
                            nc.vector.tensor_copy(
                                P_sb[:, t0:t0 + 2 * w, 0], pv)
                    else:
                        ts = stiles[r]
                        g.dma_gather(ts[:, a:a + w, :], tabs[r][:],
                                     sidx_sb[r][:, a * 8:(a + w) * 8],
                                     w * 128, w * 128, DPP,
                                     single_packet=SPKT, queue_num=q)
                        if with_pcopy:
                            t0 = rb + 2 * npb[r] + a
                            nc.vector.tensor_copy(
                                P_sb[:, t0:t0 + w, 0], ts[:, a:a + w, PCOL])

            if variant == "gather_only":
                emit_gathers()
                lg_sb = work.tile([3, BPC], F32, tag="lgs", name=f"lgs_{it}")
                nc.vector.tensor_scalar(
                    lg_sb[:], P_sb[0:3, 0:BPC, 0], 1.0, None,
                    mybir.AluOpType.mult)
                nc.sync.dma_start(out=out_d[:], in_=lg_sb[:])
                return

            def wsum(vec_t, E_t, entry, pi):
                """DoubleRow weighted sum for plan entry (slots t, t+1)."""
                kind, r, a, t = entry
                first, last = pi == 0, pi == len(plan) - 1
                if kind == "P":
                    rhs = ptiles[r][:, a, :].rearrange(
                        "p (m d) -> p m d", m=2)[:, :, 0:NE]
                else:
                    rhs = stiles[r][:, a:a + 2, 0:NE]
                nc.tensor.matmul(vec_t[:, :], lhsT=E_t[:, t:t + 2, :],
                                 rhs=rhs, start=first, stop=last,
                                 perf_mode=DROW)

            # ---- main pass: gathers + hop-1 E/PE pipeline ----
            E1 = work.tile([128, s_slots, BPC], F8, tag="E", bufs=2,
                           name=f"E1_{it}")
            vec1 = ps.tile([16, NE], F32, tag="vec", bufs=2, name=f"vec1_{it}")
            emit_gathers()
            pi = 0
            for ci_, lo in enumerate(range(0, s_slots, GRP)):
                hi = min(lo + GRP, s_slots)
                ebuild(E1, C1, lo, hi, 1, ci_)
                while pi < len(plan) and plan[pi][3] + 2 <= hi:
                    wsum(vec1, E1, plan[pi], pi)
                    pi += 1
            assert pi == len(plan)

            # ---- hop tails & remaining hops ----
            def hop_tail(vec, uT_prev, hop):
                zr = work.tile([16, 1], F32, tag="zr", name=f"zr{hop}_{it}")
                nc.vector.reciprocal(zr[:], vec[:, D:D + 1])
                vecN = work.tile([16, NE], F16, tag="vecN",
                                 name=f"vecN{hop}_{it}")
                nc.vector.tensor_scalar(vecN[:], vec[:, :], zr[:], None,
                                        mybir.AluOpType.mult)
                vNT = ps.tile([128, 3, BPC], F16, tag="mp", bufs=2,
                              name=f"vNT{hop}_{it}")
                for i, (a, b) in enumerate(CH):
                    nc.tensor.transpose(vNT[0:b - a, i, :], vecN[:, a:b],
                                        id16_sb[:])
                up = ps.tile([128, 3, BPC], F32, tag="mp", bufs=2,
                             name=f"up{hop}_{it}")
                for i, (a, b) in enumerate(CH):
                    for j, (aj, bj) in enumerate(CH):
                        nc.tensor.matmul(up[0:b - a, i, :],
                                         lhsT=wtr_sb[0:bj - aj, j, a:b],
                                         rhs=uT_prev[0:bj - aj, j, :],
                                         start=(j == 0), stop=(j == 2))
                vNs = work.tile([128, 3, BPC], F16, tag="vNs",
                                name=f"vNs{hop}_{it}")
                for i, (a, b) in enumerate(CH):
                    nc.vector.tensor_copy(vNs[0:b - a, i, :],
                                          vNT[0:b - a, i, :])
                uT_n = work.tile([128, 3, BPC], F16, tag="uT",
                                 name=f"uT{hop}_{it}")
                for i, (a, b) in enumerate(CH):
                    nc.vector.tensor_tensor(
                        out=uT_n[0:b - a, i, :], in0=up[0:b - a, i, :],
                        in1=vNs[0:b - a, i, :], op=mybir.AluOpType.add)
                    nc.vector.tensor_tensor(
                        out=uT_n[0:b - a, i, :], in0=uT_n[0:b - a, i, :],
                        in1=btr_sb[0:b - a, i, :].to_broadcast([b - a, BPC]),
                        op=mybir.AluOpType.add)
                return uT_n

            uT_cur = hop_tail(vec1, uT, 1)
            hops = () if variant == "hop1_only" else (2, 3)
            for hop in hops:
                Cm = build_C(uT_cur, hop)
                E = work.tile([128, s_slots, BPC], F8, tag="E", bufs=2,
                              name=f"E{hop}_{it}")
                vec = ps.tile([16, NE], F32, tag="vec", bufs=2,
                              name=f"vec{hop}_{it}")
                ets = []
                ECH = 56
                nch = -(-s_slots // ECH)
                for ci, lo in enumerate(range(0, s_slots, ECH)):
                    hi = min(lo + ECH, s_slots)
                    n = hi - lo
                    Et = work.tile([128, ECH, BPC], F16, tag="Eth", bufs=nch,
                                   name=f"Eth{hop}_{ci}_{it}")
                    nc.vector.tensor_tensor(
                        out=Et[:, 0:n, :],
                        in0=P_sb[:, lo:hi, :].to_broadcast([128, n, BPC]),
                        in1=Cm[:].to_broadcast([128, n, BPC]),
                        op=mybir.AluOpType.add)
                    ets.append(Et)
                pi = 0
                for ci, lo in enumerate(range(0, s_slots, ECH)):
                    hi = min(lo + ECH, s_slots)
                    n = hi - lo
                    Et = ets[ci]
                    nc.scalar.activation(Et[:, 0:n, :], Et[:, 0:n, :],
                                         ACT.Tanh)
                    nc.scalar.activation(Et[:, 0:n, :], Et[:, 0:n, :],
                                         ACT.Exp)
                    nc.vector.tensor_tensor(
                        out=E[:, lo:hi, :], in0=Et[:, 0:n, :],
                        in1=masks_sb[:, lo:hi, :], op=mybir.AluOpType.mult)
                    while pi < len(plan) and plan[pi][3] + 2 <= hi:
                        wsum(vec, E, plan[pi], pi)
                        pi += 1
                uT_cur = hop_tail(vec, uT_cur, hop)

            lg = ps.tile([3, BPC], F32, tag="mp", bufs=2, name=f"lg_{it}")
            for j, (aj, bj) in enumerate(CH):
                nc.tensor.matmul(lg[:, :], lhsT=wout_sb[0:bj - aj, j, :],
                                 rhs=uT_cur[0:bj - aj, j, :],
                                 start=(j == 0), stop=(j == 2))
            lg_sb = work.tile([3, BPC], F32, tag="lgs", name=f"lgs_{it}")
            nc.vector.tensor_tensor(
                out=lg_sb[:], in0=lg[:, :],
                in1=bout_sb[:].to_broadcast([3, BPC]),
                op=mybir.AluOpType.add)
            nc.sync.dma_start(out=out_d[:], in_=lg_sb[:])

        if loop_n is None:
            body(0)
        else:
            with tc.For_i(0, loop_n, 1):
                body(0)
    nc.compile()
    return nc


def kernel(**inputs):
    in_maps, meta = _prep(**inputs)
    nc = _build(meta, nq=4)
    res = run_bass_kernel_spmd(nc, in_maps, core_ids=list(range(NCORES)))
    out = np.zeros((B, 3), np.float32)
    for c in range(NCORES):
        out[c * BPC:(c + 1) * BPC] = res.results[c]["outl"].T
    return out
